# revision 25
# baseline (speedup 1.0000x reference)
"""Multi-head attention (B=1, S=4096, D=1024, H=16, Hd=64) on 8 Trainium2 cores.

Sharding: tensor-parallel over heads - 2 heads per core. Each core computes
q/k/v projections for its 2 heads (128 dims), flash-style attention without
max-subtraction (scores are ~N(0,1) after scaling so exp never overflows),
and a partial output projection with its 128 rows of wo. Host sums the 8
partial outputs and adds bo.

The kernel is globally PE-bound (~310us of Tensor-engine time); the
schedule is built to keep the PE streaming and everything else off its
critical path:
  - Lead-in: wk + x0 DMA first; the PE warms up (HAM K=8/8) on dummy
    matmuls during the DMA wait, so the first exp fires at ~21us instead
    of ~36us.
  - q-projections for blocks 2..7 are deferred into blocks 1..6 (only
    k/v projections are needed during block 0's pass over all k-chunks);
    all 8 x-blocks stay resident in SBUF. Block 0 is the PE-heavy phase
    (all k/v projections + its own stream); the ACT engine idles there,
    so projection bias-adds ride on ACT during block 0.
  - The exp stream stages scores in two engine-segregated PSUM rings:
    slices 0-1 of each 3-slice group in a [128,2,512] ring consumed by
    ACT, slice 2 in a [128,512] ring consumed mid-block by the DVE as a
    one-op Schraudolph exp in the fp16 bit domain:
    int16(round(s*(2^10*log2e)/8 + (15-c)*2^10)) bitcast to fp16, ~3%
    sawtooth error (final rel err ~6e-3 vs 2e-2 budget; HW convert is
    round-to-nearest). Ring segregation keeps ACT slot reuse decoupled
    from the DVE queue. Scores are emitted in 2-group bursts to halve
    score->ctx PE transitions.
  - Normalize orders denominator work first (sums on ACT -> broadcast
    matmul -> reciprocal) so the ctx PSUM ring turns over early; the h1
    half merges into cs01 via SBUF DMA (DVE has no cross-lane path),
    except in the last block where the out-projection splits into two
    K=64 matmuls (wo rows 64-127 duplicated at partitions 0-63) to skip
    the merge latency. Denominator broadcasts run in fp16.
  - The last block's out-proj pieces cycle over 4 psum rings.
Best measured: 354us (full clock; the chip sometimes runs ~15-20%
downclocked - check steady-state EXP duration ~1045ns before comparing).

Layouts on device (per core):
  xT   [8, 128, 512] fp16 per block: partitions = d-chunk dims
  qT/kT[128, S] fp16   partitions = head dims (h0: 0-63, h1: 64-127)
  v4   [128, 2, 65] fp16 per k-chunk: partitions = seq rows, col 64 = ones
  scores psum [128 (k rows), 3x512 (q)] fp32 -> exp on ACT/DVE -> ex fp16
  ctx^T psum [65, 512] fp32 per head, accumulated over 32 k-chunks
  out  [S, D] fp16 partials, summed + bo on host
"""

import os
import sys
import types

import numpy as np

S = 4096
D = 1024
H = 16
HD = 64
N_CORES = 8
HPC = H // N_CORES  # heads per core = 2
DC = D // 128       # d-chunks = 8
QB = 512            # q block

# fp16 Schraudolph constants: exp(s*scale) ~= bitcast_f16(int16(round(
#   s*scale*log2(e)*2^10 + (15-c)*2^10))), c tuned for min max-rel-err.
SCHRAU_C = 0.0430

_LAST_EXEC_NS = None


def _install_ntff_hook_shim():
    if "antenv.axon_hooks" in sys.modules:
        return
    try:
        import antenv
        from trn_agent_boot.trn_boot import _ntff_profile_via_ctypes

        hook = _ntff_profile_via_ctypes("/opt/axon/libaxon_pjrt.so")
    except Exception:
        return
    mod = types.ModuleType("antenv.axon_hooks")
    _state = {"hook": hook}
    mod.get_axon_ntff_profile_hook = lambda: _state["hook"]
    mod.set_axon_ntff_profile_hook = lambda h: _state.update(hook=h)
    sys.modules["antenv.axon_hooks"] = mod
    antenv.axon_hooks = mod


def _build(s=S):
    import concourse.bass as bass
    import concourse.mybir as mybir
    import concourse.tile as tile
    from concourse import bacc

    f32 = mybir.dt.float32
    f16 = mybir.dt.float16
    i16 = mybir.dt.int16
    Exp = mybir.ActivationFunctionType.Exp
    ADD = mybir.AluOpType.add
    MULT = mybir.AluOpType.mult

    KC = s // 128     # k-chunks
    PB = 512          # projection block
    NP = s // PB      # projection / q blocks
    QB = 512
    GS = 3            # (kc, h) slices per exp staging group
    LOOK = 2          # score groups emitted ahead of ctx

    nc = bacc.Bacc("TRN2", target_bir_lowering=False, debug=False,
                   num_devices=N_CORES)

    xT_d = nc.declare_dram_parameter("xT", [NP, 128, DC, 512], f16,
                                     isOutput=False)
    wq_d = nc.declare_dram_parameter("wq", [128, D], f16, isOutput=False)
    wk_d = nc.declare_dram_parameter("wk", [128, D], f16, isOutput=False)
    wv_d = nc.declare_dram_parameter("wv", [128, DC, 130], f16, isOutput=False)
    bq_d = nc.declare_dram_parameter("bq", [128, 1], f32, isOutput=False)
    bk_d = nc.declare_dram_parameter("bk", [128, 1], f32, isOutput=False)
    bvb_d = nc.declare_dram_parameter("bvb", [128, 2, 65], f16, isOutput=False)
    wo_d = nc.declare_dram_parameter("wo", [128, D], f16, isOutput=False)
    wo2_d = nc.declare_dram_parameter("wo2", [64, D], f16, isOutput=False)
    out_d = nc.declare_dram_parameter("out", [s, D], f16, isOutput=True)

    with tile.TileContext(nc) as tc:
        import contextlib
        with contextlib.ExitStack() as ctx:
            wpool = ctx.enter_context(tc.tile_pool(name="w", bufs=1))
            xpool = ctx.enter_context(tc.tile_pool(name="x", bufs=NP))
            kpool = ctx.enter_context(tc.tile_pool(name="kt", bufs=1))
            qpool = ctx.enter_context(tc.tile_pool(name="qt", bufs=NP))
            vpool = ctx.enter_context(tc.tile_pool(name="v4", bufs=KC))
            epool = ctx.enter_context(tc.tile_pool(name="ex", bufs=8))
            cpool = ctx.enter_context(tc.tile_pool(name="ctxs", bufs=2))
            spool = ctx.enter_context(tc.tile_pool(name="sums", bufs=2))
            opool = ctx.enter_context(tc.tile_pool(name="outs", bufs=4))
            tpool = ctx.enter_context(tc.tile_pool(name="tmp", bufs=2))
            # PSUM: stage ring 2x3 banks + ctx0 + ctx1 = 8 banks.
            stg = ctx.enter_context(tc.tile_pool(name="stg", bufs=2, space="PSUM"))
            cp = ctx.enter_context(tc.tile_pool(name="cp", bufs=1, space="PSUM"))

            # ---- constants / weights ----
            wq_t = wpool.tile([128, D], f16, tag="wq")
            wk_t = wpool.tile([128, D], f16, tag="wk")
            wv_t = wpool.tile([128, DC, 130], f16, tag="wv")
            wo_t = wpool.tile([128, D], f16, tag="wo")
            bq_t = wpool.tile([128, 1], f32, tag="bq")
            bk_t = wpool.tile([128, 1], f32, tag="bk")
            bvb_t = wpool.tile([128, 2, 65], f16, tag="bvb")
            ones_t = wpool.tile([65, 64], f16, tag="ones")
            wo2_t = wpool.tile([64, D], f16, tag="wo2")
            warm_t = wpool.tile([128, 512], f16, tag="warm")

            x_tiles = [None] * NP

            def fetch_x(b):
                xb = xpool.tile([128, DC, PB], f16, tag="xb")
                nc.sync.dma_start(xb[:], xT_d[b])
                x_tiles[b] = xb

            # DMA order: what block 0 needs first, then x blocks in
            # consumption order, wo (first needed at block-0's epilogue,
            # ~70us in) last.
            nc.sync.dma_start(wk_t[:], wk_d[:])
            fetch_x(0)
            nc.sync.dma_start(wq_t[:], wq_d[:])
            nc.sync.dma_start(bk_t[:], bk_d[:])
            nc.sync.dma_start(bq_t[:], bq_d[:])
            nc.sync.dma_start(wv_t[:], wv_d[:])
            nc.sync.dma_start(bvb_t[:], bvb_d[:])
            for b in range(1, NP):
                fetch_x(b)
            nc.sync.dma_start(wo_t[:], wo_d[:])
            nc.sync.dma_start(wo2_t[:], wo2_d[:])
            nc.vector.memset(ones_t[:], 1.0)
            nc.vector.memset(warm_t[:], 0.0)

            kT = kpool.tile([128, s], f16, tag="kT")
            q_tiles = [None] * NP
            v_tiles = [None] * KC

            def mm(out, lhsT, rhs, start, stop, tile_position=None):
                return nc.tensor.matmul(out, lhsT, rhs, start=start,
                                        stop=stop, tile_position=tile_position)

            # ---- PE warm-up: ~12 dummy matmuls flip HAM to 8/8 during the
            # DMA wait so the first projections run at 2.4GHz.
            for w in range(10):
                ps = stg.tile([128, 512], f32, tag="stage")
                mm(ps[:], warm_t[:, 0:128], warm_t[:], start=True, stop=True)

            # ---- projection emitters ----
            def emit_kproj(b, eng="dve"):
                xb = x_tiles[b]
                ps = stg.tile([128, PB], f32, tag="stgB")
                for c in range(DC):
                    mm(ps[:], wk_t[:, c * 128:(c + 1) * 128], xb[:, c, :],
                       start=(c == 0), stop=(c == DC - 1))
                dst = kT[:, b * PB:(b + 1) * PB]
                if eng == "act":
                    nc.scalar.add(dst, ps[:], bk_t[:])
                else:
                    nc.vector.tensor_scalar_add(dst, ps[:], bk_t[:])

            def emit_qproj(b, eng="dve"):
                qb = qpool.tile([128, PB], f16, tag="qT")
                ps = stg.tile([128, PB], f32, tag="stgB")
                for c in range(DC):
                    mm(ps[:], wq_t[:, c * 128:(c + 1) * 128], xb_of(b)[:, c, :],
                       start=(c == 0), stop=(c == DC - 1))
                if eng == "act":
                    nc.scalar.add(qb[:], ps[:], bq_t[:])
                else:
                    nc.vector.tensor_scalar_add(qb[:], ps[:], bq_t[:])
                q_tiles[b] = qb

            def xb_of(b):
                return x_tiles[b]

            # deferred q-projection, split in halves so no stage slot is
            # held longer than ~2us inside the ACT-paced steady blocks.
            qtmp_box = {}

            def emit_qproj_h1(b):
                ps = stg.tile([128, PB], f32, tag="stgB")
                for c in range(4):
                    mm(ps[:], wq_t[:, c * 128:(c + 1) * 128], x_tiles[b][:, c, :],
                       start=(c == 0), stop=(c == 3))
                qtmp = tpool.tile([128, PB], f32, tag="qtmp")
                nc.vector.tensor_scalar_add(qtmp[:], ps[:], bq_t[:])
                qtmp_box[b] = qtmp

            def emit_qproj_h2(b):
                ps = stg.tile([128, PB], f32, tag="stgB")
                for c in range(4, DC):
                    mm(ps[:], wq_t[:, c * 128:(c + 1) * 128], x_tiles[b][:, c, :],
                       start=(c == 4), stop=(c == DC - 1))
                qb = qpool.tile([128, PB], f16, tag="qT")
                nc.vector.scalar_tensor_tensor(
                    qb[:], ps[:], 0.0, qtmp_box.pop(b)[:], ADD, ADD)
                q_tiles[b] = qb

            def emit_v(b, j):
                xb = x_tiles[b]
                kc = b * 4 + j
                vps = stg.tile([128, 130], f32, tag="stgB")
                for c in range(DC):
                    mm(vps[:], xb[:, c, j * 128:(j + 1) * 128],
                       wv_t[:, c, :], start=(c == 0), stop=(c == DC - 1))
                v4 = vpool.tile([128, 2, 65], f16, tag="v4")
                nc.vector.tensor_add(
                    v4[:], vps[:].rearrange("p (h m) -> p h m", h=2),
                    bvb_t[:])
                v_tiles[kc] = v4

            # ---- attention stream plumbing ----
            slices = [(kc, h) for kc in range(KC) for h in range(2)]
            groups = [slices[i:i + GS] for i in range(0, len(slices), GS)]
            NG = len(groups)
            items = [(b, gi) for b in range(NP) for gi in range(NG)]

            EXP_SCALE = float(1.0 / np.sqrt(HD))
            DVE_C0 = float(1024.0 * np.log2(np.e) * EXP_SCALE)
            DVE_C1 = float((15.0 - SCHRAU_C) * 1024.0)

            def emit_scores_exp(b, gi, i):
                # slices 0-1 stage in the ACT ring (stage), slice 2 in its
                # own ring (stgB) consumed by either ACT or DVE - the rings
                # decouple so ACT's slot reuse never waits on the DVE queue.
                grp = groups[gi]
                ns = len(grp)
                na = min(ns, 2)
                qb = q_tiles[b]
                st = stg.tile([128, 2, QB], f32, tag="stage")
                ex = epool.tile([128, GS, QB], f16, tag="ex")
                for slot in range(na):
                    kc, h = grp[slot]
                    mm(st[:, slot, :],
                       kT[h * 64:(h + 1) * 64, kc * 128:(kc + 1) * 128],
                       qb[h * 64:(h + 1) * 64, :],
                       start=True, stop=True)
                stb = None
                if ns == 3:
                    kc, h = grp[2]
                    stb = stg.tile([128, QB], f32, tag="stgB")
                    mm(stb[:],
                       kT[h * 64:(h + 1) * 64, kc * 128:(kc + 1) * 128],
                       qb[h * 64:(h + 1) * 64, :],
                       start=True, stop=True)
                nc.scalar.activation(
                    ex[:, 0:na, :], st[:, 0:na, :], Exp,
                    bias=0.0, scale=EXP_SCALE)
                if ns == 3:
                    # DVE takes the third slice mid-block in steady blocks
                    # (its queue is clumped with normalize work near the
                    # block boundaries); ACT takes it otherwise.
                    off = (i >= NG) and (2 <= gi < NG - 2)
                    if off:
                        nc.vector.tensor_scalar(
                            ex[:, 2, :].bitcast(i16), stb[:],
                            DVE_C0, DVE_C1, MULT, ADD)
                    else:
                        nc.scalar.activation(
                            ex[:, 2, :], stb[:], Exp,
                            bias=0.0, scale=EXP_SCALE)
                return ex

            # normalize block b's ctx accumulators -> cs01 (frees cp ring)
            def emit_normalize(b, ctxp0, ctxp1):
                # h0 normalized in cs01[0:64]; h1 in its own base-0 tile
                # (DVE has no cross-lane path), then DMA'd into cs01[64:].
                cs01 = cpool.tile([128, QB], f16, tag="cs01")
                cs1t = cpool.tile([64, QB], f16, tag="cs1t")
                sums = spool.tile([65, 2 * QB], f16, tag="sums")
                # ACT: denominator rows first (they gate the rb broadcast,
                # whose reciprocal frees the cp ring for block b+1), then
                # the big h0 copy. DVE: h1 copy, then recips BEFORE muls so
                # the cp ring turns over as early as possible.
                nc.scalar.copy(sums[64:65, 0:QB], ctxp0[64:65, :])
                nc.scalar.copy(sums[64:65, QB:2 * QB], ctxp1[64:65, :])
                nc.scalar.copy(cs01[0:64, :], ctxp0[0:64, :])
                nc.vector.tensor_copy(cs1t[:], ctxp1[0:64, :])
                rb0 = cp.tile([64, QB], f32, tag="ctx0")
                mm(rb0[:], ones_t[64:65, :],
                   sums[64:65, 0:QB], start=True, stop=True)
                rb1 = cp.tile([64, QB], f32, tag="ctx1")
                mm(rb1[:], ones_t[64:65, :],
                   sums[64:65, QB:2 * QB], start=True, stop=True)
                rec = spool.tile([64, 2, QB], f32, tag="rec")
                nc.vector.reciprocal_approx_fast(rec[:, 0, :], rb0[:])
                nc.vector.reciprocal_approx_fast(rec[:, 1, :], rb1[:])
                nc.vector.tensor_mul(cs1t[:], cs1t[:], rec[:, 1, :])
                if b < NP - 1:
                    nc.sync.dma_start(cs01[64:128, :], cs1t[:])
                nc.vector.tensor_mul(cs01[0:64, :], cs01[0:64, :],
                                     rec[:, 0, :])
                cs1_box[0] = cs1t
                return cs01

            # one out-proj piece: out[Q*QB + m*128 ... , nh*512 ...]
            TAIL_TAGS = ["stage", "ctx0", "stgB", "ctx1"]
            cs1_box = [None]

            def emit_out_piece(b, cs01, m, nh, pi=0):
                tag = TAIL_TAGS[pi % 4] if b == NP - 1 else "stgB"
                pool = stg if tag in ("stage", "stgB") else cp
                op = pool.tile([128, 512], f32, tag=tag)
                if b == NP - 1:
                    # split per head: no cross-partition cs merge needed
                    mm(op[:], cs01[0:64, m * 128:(m + 1) * 128],
                       wo_t[0:64, nh * 512:(nh + 1) * 512],
                       start=True, stop=False)
                    mm(op[:], cs1_box[0][:, m * 128:(m + 1) * 128],
                       wo2_t[:, nh * 512:(nh + 1) * 512],
                       start=False, stop=True)
                else:
                    mm(op[:], cs01[:, m * 128:(m + 1) * 128],
                       wo_t[:, nh * 512:(nh + 1) * 512], start=True, stop=True)
                ob = opool.tile([128, 512], f16, tag="ob")
                if b == NP - 1 and (m + nh) % 2 == 0:
                    # tail: ACT is done with exp; steal it for half the casts
                    nc.scalar.copy(ob[:], op[:])
                else:
                    nc.vector.tensor_copy(ob[:], op[:])
                nc.sync.dma_start(
                    out_d[b * QB + m * 128:b * QB + (m + 1) * 128,
                          nh * 512:(nh + 1) * 512],
                    ob[:])

            # ---- phase A: block-0 k/q-proj up front; the first two score
            # groups go out before kproj(1) so the exp stream starts ASAP.
            emit_kproj(0, "act")
            emit_qproj(0, "dve")

            # filler schedule: {global ctx iteration: [(when, fn), ...]}
            # when: "pre" runs before the score emission of that iteration
            # (needed for kT producers), "post" runs after it (v tiles etc.)
            fillers = {}

            def add_filler(i, fn, when="pre"):
                fillers.setdefault(i, []).append((when, fn))

            def E(fn, *a, **kw):
                return lambda: fn(*a, **kw)

            # block-0 / block-1 remaining projections. kproj(1) is a pre
            # filler of iteration 0: emitted after the first two score
            # groups (phase A) but before group 2 (which touches kc 4).
            add_filler(0, E(emit_kproj, 1, "act"))
            add_filler(0, E(emit_v, 0, 0), "post")
            add_filler(0, E(emit_v, 0, 1), "post")
            add_filler(1, E(emit_v, 0, 2), "post")
            add_filler(1, E(emit_v, 0, 3), "post")
            add_filler(2, E(emit_qproj, 1, "dve"), "post")
            add_filler(2, E(emit_v, 1, 0), "post")
            add_filler(3, E(emit_v, 1, 1), "post")
            add_filler(3, E(emit_v, 1, 2), "post")
            add_filler(4, E(emit_v, 1, 3), "post")

            # k/v projections of blocks 2..7 during block 0, on their
            # score-frontier deadlines: k-proj(p) must be emitted before the
            # frontier (i + LOOK, group (8p)//3) first touches chunk 4p.
            for p in range(2, NP):
                # scores for group g are emitted at the even iteration
                # i with i+2 <= g <= i+3, so kproj(p) (a pre filler) must
                # land at or before 2*((g-2)//2) for g = (8p)//3.
                base = min(3 * (p - 2) + 4, 2 * (((8 * p) // 3 - 2) // 2))
                add_filler(base, E(emit_kproj, p, "act"))
                for jj in range(4):
                    add_filler(base + 1 + (jj // 2), E(emit_v, p, jj), "post")

            # deferred q-projections: q(p) computed during block p-1; both
            # halves in one iteration (2 stage tiles) to keep ring parity.
            for p in range(2, NP):
                add_filler((p - 1) * NG + 3, E(emit_qproj_h1, p), "post")
                add_filler((p - 1) * NG + 9, E(emit_qproj_h2, p), "post")

            # ---- the flat stream ----
            ex_store = {}
            jbox = [0]

            def emit_scores_upto(lim):
                j = jbox[0]
                while j < len(items) and j <= lim:
                    ex_store[j] = emit_scores_exp(*items[j], j)
                    j += 1
                jbox[0] = j

            emit_scores_upto(LOOK - 1)  # first exps before kproj(1)

            pend_out = []  # deferred out-proj pieces of the previous block
            ctxp0 = ctxp1 = None
            for i, (b, gi) in enumerate(items):
                if gi == 0:
                    ctxp0 = cp.tile([65, QB], f32, tag="ctx0")
                    ctxp1 = cp.tile([65, QB], f32, tag="ctx1")
                pre = [f for w, f in fillers.get(i, ()) if w == "pre"]
                post = [f for w, f in fillers.pop(i, ()) if w == "post"]
                for fn in pre:
                    fn()
                if i % 2 == 0:
                    emit_scores_upto(i + LOOK + 1)
                for fn in post:
                    fn()
                # deferred epilogue pieces of the previous q-block,
                # drained in PAIRS so the stage-ring parity of the score
                # stream is preserved.
                if pend_out and gi >= 2 and gi % 2 == 0:
                    pb, pcs, pm, pnh = pend_out.pop(0)
                    emit_out_piece(pb, pcs, pm, pnh)
                # ctx accumulation for group gi
                ex = ex_store.pop(i)
                for slot, (kc, h) in enumerate(groups[gi]):
                    ctxp = ctxp0 if h == 0 else ctxp1
                    mm(ctxp[:], v_tiles[kc][:, h, :], ex[:, slot, :],
                       start=(kc == 0), stop=(kc == KC - 1))
                if gi == NG - 1:
                    # normalize now (frees ctx ring for b+1); out-proj
                    # pieces trail into the next block's groups.
                    cs01 = emit_normalize(b, ctxp0, ctxp1)
                    pieces = [(b, cs01, m, nh)
                              for m in range(QB // 128)
                              for nh in range(D // 512)]
                    if b + 1 < NP:
                        pend_out.extend(pieces)
                    else:
                        for pi, (pb, pcs, pm, pnh) in enumerate(pieces):
                            emit_out_piece(pb, pcs, pm, pnh, pi)
            # flush any stragglers
            for pb, pcs, pm, pnh in pend_out:
                emit_out_piece(pb, pcs, pm, pnh)

    nc.compile()
    return nc


def _shard_inputs(x, wq, bq, wk, bk, wv, bv, wo, bo, s):
    npdt16 = np.float16
    # [D, s] -> contiguous per-block layout [s//512, 128, D//128, 512]
    xT2 = np.asarray(x, np.float32).reshape(s, D).T
    xT = np.ascontiguousarray(
        xT2.reshape(D // 128, 128, s // 512, 512).transpose(2, 1, 0, 3)
    ).astype(npdt16)

    def lhsT_layout(w, c):
        blk = np.asarray(w, np.float32)[:, c * 128:(c + 1) * 128]
        return np.ascontiguousarray(
            blk.reshape(DC, 128, 128).transpose(1, 0, 2).reshape(128, D)
        ).astype(npdt16)

    def wv_aug_layout(w, c):
        # [128, DC, 130]: per d-chunk, [h0 cols | 0 | h1 cols | 0]
        blk = np.asarray(w, np.float32)[:, c * 128:(c + 1) * 128]  # [D, 128]
        aug = np.zeros((DC, 128, 130), np.float32)
        aug[:, :, 0:64] = blk[:, 0:64].reshape(DC, 128, 64)
        aug[:, :, 65:129] = blk[:, 64:128].reshape(DC, 128, 64)
        return np.ascontiguousarray(aug.transpose(1, 0, 2)).astype(npdt16)

    def bvb_layout(bv, c):
        # [128, 2, 65]: v bias broadcast over k-rows + ones column
        bvc = np.asarray(bv, np.float32)[c * 128:(c + 1) * 128]
        t = np.empty((2, 65), np.float32)
        t[0, 0:64] = bvc[0:64]
        t[1, 0:64] = bvc[64:128]
        t[:, 64] = 1.0
        return np.ascontiguousarray(
            np.broadcast_to(t, (128, 2, 65))).astype(npdt16)

    in_maps = []
    for c in range(N_CORES):
        in_maps.append({
            "xT": xT,
            "wq": lhsT_layout(wq, c),
            "wk": lhsT_layout(wk, c),
            "wv": wv_aug_layout(wv, c),
            "bq": np.ascontiguousarray(
                np.asarray(bq, np.float32)[c * 128:(c + 1) * 128, None]),
            "bk": np.ascontiguousarray(
                np.asarray(bk, np.float32)[c * 128:(c + 1) * 128, None]),
            "bvb": bvb_layout(bv, c),
            "wo": np.ascontiguousarray(
                np.asarray(wo, np.float32)[c * 128:(c + 1) * 128, :]
            ).astype(npdt16),
            "wo2": np.ascontiguousarray(
                np.asarray(wo, np.float32)[c * 128 + 64:(c + 1) * 128, :]
            ).astype(npdt16),
        })
    return in_maps


def run(x, wq, bq, wk, bk, wv, bv, wo, bo, trace=False, s=S):
    global _LAST_EXEC_NS
    from concourse.bass_utils import run_bass_kernel_spmd

    if trace:
        _install_ntff_hook_shim()
    nc = _build(s)
    in_maps = _shard_inputs(x, wq, bq, wk, bk, wv, bv, wo, bo, s)
    res = run_bass_kernel_spmd(nc, in_maps, core_ids=list(range(N_CORES)),
                               trace=trace)
    _LAST_EXEC_NS = res.exec_time_ns
    out = res.results[0]["out"].astype(np.float64)
    for c in range(1, N_CORES):
        out += res.results[c]["out"]
    out += np.asarray(bo, np.float64)
    return out.astype(np.float32).reshape(1, s, D)


def kernel(x, wq, bq, wk, bk, wv, bv, wo, bo):
    trace = bool(os.environ.get("BASS_MHA_TRACE"))
    return run(x, wq, bq, wk, bk, wv, bv, wo, bo, trace=trace)


# revision 26
# speedup vs baseline: 1.0037x; 1.0037x over previous
"""Multi-head attention (B=1, S=4096, D=1024, H=16, Hd=64) on 8 Trainium2 cores.

Sharding: tensor-parallel over heads - 2 heads per core. Each core computes
q/k/v projections for its 2 heads (128 dims), flash-style attention without
max-subtraction (scores are ~N(0,1) after scaling so exp never overflows),
and a partial output projection with its 128 rows of wo. Host sums the 8
partial outputs and adds bo.

The kernel is globally PE-bound (~310us of Tensor-engine time); the
schedule is built to keep the PE streaming and everything else off its
critical path:
  - Lead-in: wk + x0 DMA first; the PE warms up (HAM K=8/8) on dummy
    matmuls during the DMA wait, so the first exp fires at ~21us instead
    of ~36us.
  - q-projections for blocks 2..7 are deferred into blocks 1..6 (only
    k/v projections are needed during block 0's pass over all k-chunks);
    all 8 x-blocks stay resident in SBUF. Block 0 is the PE-heavy phase
    (all k/v projections + its own stream); the ACT engine idles there,
    so projection bias-adds ride on ACT during block 0.
  - The exp stream stages scores in two engine-segregated PSUM rings:
    slices 0-1 of each 3-slice group in a [128,2,512] ring consumed by
    ACT, slice 2 in a [128,512] ring consumed mid-block by the DVE as a
    one-op Schraudolph exp in the fp16 bit domain:
    int16(round(s*(2^10*log2e)/8 + (15-c)*2^10)) bitcast to fp16, ~3%
    sawtooth error (final rel err ~6e-3 vs 2e-2 budget; HW convert is
    round-to-nearest). Ring segregation keeps ACT slot reuse decoupled
    from the DVE queue. Scores are emitted in 2-group bursts to halve
    score->ctx PE transitions.
  - Normalize orders denominator work first (sums on ACT -> broadcast
    matmul -> reciprocal) so the ctx PSUM ring turns over early; the h1
    half merges into cs01 via SBUF DMA (DVE has no cross-lane path),
    except in the last block where the out-projection splits into two
    K=64 matmuls (wo rows 64-127 duplicated at partitions 0-63) to skip
    the merge latency. Denominator broadcasts run in fp16.
  - The last block's out-proj pieces cycle over 4 psum rings.
Best measured: 354us (full clock; the chip sometimes runs ~15-20%
downclocked - check steady-state EXP duration ~1045ns before comparing).

Layouts on device (per core):
  xT   [8, 128, 512] fp16 per block: partitions = d-chunk dims
  qT/kT[128, S] fp16   partitions = head dims (h0: 0-63, h1: 64-127)
  v4   [128, 2, 65] fp16 per k-chunk: partitions = seq rows, col 64 = ones
  scores psum [128 (k rows), 3x512 (q)] fp32 -> exp on ACT/DVE -> ex fp16
  ctx^T psum [65, 512] fp32 per head, accumulated over 32 k-chunks
  out  [S, D] fp16 partials, summed + bo on host
"""

import os
import sys
import types

import numpy as np

S = 4096
D = 1024
H = 16
HD = 64
N_CORES = 8
HPC = H // N_CORES  # heads per core = 2
DC = D // 128       # d-chunks = 8
QB = 512            # q block

# fp16 Schraudolph constants: exp(s*scale) ~= bitcast_f16(int16(round(
#   s*scale*log2(e)*2^10 + (15-c)*2^10))), c tuned for min max-rel-err.
SCHRAU_C = 0.0430

_LAST_EXEC_NS = None


def _install_ntff_hook_shim():
    if "antenv.axon_hooks" in sys.modules:
        return
    try:
        import antenv
        from trn_agent_boot.trn_boot import _ntff_profile_via_ctypes

        hook = _ntff_profile_via_ctypes("/opt/axon/libaxon_pjrt.so")
    except Exception:
        return
    mod = types.ModuleType("antenv.axon_hooks")
    _state = {"hook": hook}
    mod.get_axon_ntff_profile_hook = lambda: _state["hook"]
    mod.set_axon_ntff_profile_hook = lambda h: _state.update(hook=h)
    sys.modules["antenv.axon_hooks"] = mod
    antenv.axon_hooks = mod


def _build(s=S):
    import concourse.bass as bass
    import concourse.mybir as mybir
    import concourse.tile as tile
    from concourse import bacc

    f32 = mybir.dt.float32
    f16 = mybir.dt.float16
    i16 = mybir.dt.int16
    Exp = mybir.ActivationFunctionType.Exp
    ADD = mybir.AluOpType.add
    MULT = mybir.AluOpType.mult

    KC = s // 128     # k-chunks
    PB = 512          # projection block
    NP = s // PB      # projection / q blocks
    QB = 512
    GS = 3            # (kc, h) slices per exp staging group
    LOOK = 2          # score groups emitted ahead of ctx

    nc = bacc.Bacc("TRN2", target_bir_lowering=False, debug=False,
                   num_devices=N_CORES)

    xT_d = nc.declare_dram_parameter("xT", [NP, 128, DC, 512], f16,
                                     isOutput=False)
    wq_d = nc.declare_dram_parameter("wq", [128, D], f16, isOutput=False)
    wk_d = nc.declare_dram_parameter("wk", [128, D], f16, isOutput=False)
    wv_d = nc.declare_dram_parameter("wv", [128, DC, 130], f16, isOutput=False)
    bq_d = nc.declare_dram_parameter("bq", [128, 1], f32, isOutput=False)
    bk_d = nc.declare_dram_parameter("bk", [128, 1], f32, isOutput=False)
    bvb_d = nc.declare_dram_parameter("bvb", [128, 2, 65], f16, isOutput=False)
    wo_d = nc.declare_dram_parameter("wo", [128, D], f16, isOutput=False)
    wo2_d = nc.declare_dram_parameter("wo2", [64, D], f16, isOutput=False)
    out_d = nc.declare_dram_parameter("out", [s, D], f16, isOutput=True)

    with tile.TileContext(nc) as tc:
        import contextlib
        with contextlib.ExitStack() as ctx:
            wpool = ctx.enter_context(tc.tile_pool(name="w", bufs=1))
            xpool = ctx.enter_context(tc.tile_pool(name="x", bufs=NP))
            kpool = ctx.enter_context(tc.tile_pool(name="kt", bufs=1))
            qpool = ctx.enter_context(tc.tile_pool(name="qt", bufs=NP))
            vpool = ctx.enter_context(tc.tile_pool(name="v4", bufs=KC))
            epool = ctx.enter_context(tc.tile_pool(name="ex", bufs=8))
            cpool = ctx.enter_context(tc.tile_pool(name="ctxs", bufs=2))
            spool = ctx.enter_context(tc.tile_pool(name="sums", bufs=2))
            opool = ctx.enter_context(tc.tile_pool(name="outs", bufs=4))
            tpool = ctx.enter_context(tc.tile_pool(name="tmp", bufs=2))
            # PSUM: stage ring 2x3 banks + ctx0 + ctx1 = 8 banks.
            stg = ctx.enter_context(tc.tile_pool(name="stg", bufs=2, space="PSUM"))
            cp = ctx.enter_context(tc.tile_pool(name="cp", bufs=1, space="PSUM"))

            # ---- constants / weights ----
            wq_t = wpool.tile([128, D], f16, tag="wq")
            wk_t = wpool.tile([128, D], f16, tag="wk")
            wv_t = wpool.tile([128, DC, 130], f16, tag="wv")
            wo_t = wpool.tile([128, D], f16, tag="wo")
            bq_t = wpool.tile([128, 1], f32, tag="bq")
            bk_t = wpool.tile([128, 1], f32, tag="bk")
            bvb_t = wpool.tile([128, 2, 65], f16, tag="bvb")
            ones_t = wpool.tile([65, 64], f16, tag="ones")
            wo2_t = wpool.tile([64, D], f16, tag="wo2")
            warm_t = wpool.tile([128, 512], f16, tag="warm")

            x_tiles = [None] * NP

            def fetch_x(b):
                xb = xpool.tile([128, DC, PB], f16, tag="xb")
                nc.sync.dma_start(xb[:], xT_d[b])
                x_tiles[b] = xb

            # DMA order: what block 0 needs first, then x blocks in
            # consumption order, wo (first needed at block-0's epilogue,
            # ~70us in) last.
            nc.sync.dma_start(wk_t[:], wk_d[:])
            fetch_x(0)
            nc.sync.dma_start(wq_t[:], wq_d[:])
            nc.sync.dma_start(bk_t[:], bk_d[:])
            nc.sync.dma_start(bq_t[:], bq_d[:])
            nc.sync.dma_start(wv_t[:], wv_d[:])
            nc.sync.dma_start(bvb_t[:], bvb_d[:])
            for b in range(1, NP):
                fetch_x(b)
            nc.sync.dma_start(wo_t[:], wo_d[:])
            nc.sync.dma_start(wo2_t[:], wo2_d[:])
            nc.vector.memset(ones_t[:], 1.0)
            nc.vector.memset(warm_t[:], 0.0)

            kT = kpool.tile([128, s], f16, tag="kT")
            q_tiles = [None] * NP
            v_tiles = [None] * KC

            def mm(out, lhsT, rhs, start, stop, tile_position=None):
                return nc.tensor.matmul(out, lhsT, rhs, start=start,
                                        stop=stop, tile_position=tile_position)

            # ---- PE warm-up: ~12 dummy matmuls flip HAM to 8/8 during the
            # DMA wait so the first projections run at 2.4GHz.
            for w in range(10):
                ps = stg.tile([128, 512], f32, tag="stage")
                mm(ps[:], warm_t[:, 0:128], warm_t[:], start=True, stop=True)

            # ---- projection emitters ----
            def emit_kproj(b, eng="dve"):
                xb = x_tiles[b]
                ps = stg.tile([128, PB], f32, tag="stgB")
                for c in range(DC):
                    mm(ps[:], wk_t[:, c * 128:(c + 1) * 128], xb[:, c, :],
                       start=(c == 0), stop=(c == DC - 1))
                dst = kT[:, b * PB:(b + 1) * PB]
                if eng == "act":
                    nc.scalar.add(dst, ps[:], bk_t[:])
                else:
                    nc.vector.tensor_scalar_add(dst, ps[:], bk_t[:])

            def emit_qproj(b, eng="dve"):
                qb = qpool.tile([128, PB], f16, tag="qT")
                ps = stg.tile([128, PB], f32, tag="stgB")
                for c in range(DC):
                    mm(ps[:], wq_t[:, c * 128:(c + 1) * 128], xb_of(b)[:, c, :],
                       start=(c == 0), stop=(c == DC - 1))
                if eng == "act":
                    nc.scalar.add(qb[:], ps[:], bq_t[:])
                else:
                    nc.vector.tensor_scalar_add(qb[:], ps[:], bq_t[:])
                q_tiles[b] = qb

            def xb_of(b):
                return x_tiles[b]

            # deferred q-projection, split in halves so no stage slot is
            # held longer than ~2us inside the ACT-paced steady blocks.
            qtmp_box = {}

            def emit_qproj_h1(b):
                ps = stg.tile([128, PB], f32, tag="stgB")
                for c in range(4):
                    mm(ps[:], wq_t[:, c * 128:(c + 1) * 128], x_tiles[b][:, c, :],
                       start=(c == 0), stop=(c == 3))
                qtmp = tpool.tile([128, PB], f32, tag="qtmp")
                nc.vector.tensor_scalar_add(qtmp[:], ps[:], bq_t[:])
                qtmp_box[b] = qtmp

            def emit_qproj_h2(b):
                ps = stg.tile([128, PB], f32, tag="stgB")
                for c in range(4, DC):
                    mm(ps[:], wq_t[:, c * 128:(c + 1) * 128], x_tiles[b][:, c, :],
                       start=(c == 4), stop=(c == DC - 1))
                qb = qpool.tile([128, PB], f16, tag="qT")
                nc.vector.scalar_tensor_tensor(
                    qb[:], ps[:], 0.0, qtmp_box.pop(b)[:], ADD, ADD)
                q_tiles[b] = qb

            def emit_v(b, j):
                xb = x_tiles[b]
                kc = b * 4 + j
                vps = stg.tile([128, 130], f32, tag="stgB")
                for c in range(DC):
                    mm(vps[:], xb[:, c, j * 128:(j + 1) * 128],
                       wv_t[:, c, :], start=(c == 0), stop=(c == DC - 1))
                v4 = vpool.tile([128, 2, 65], f16, tag="v4")
                nc.vector.tensor_add(
                    v4[:], vps[:].rearrange("p (h m) -> p h m", h=2),
                    bvb_t[:])
                v_tiles[kc] = v4

            # ---- attention stream plumbing ----
            slices = [(kc, h) for kc in range(KC) for h in range(2)]
            groups = [slices[i:i + GS] for i in range(0, len(slices), GS)]
            NG = len(groups)
            items = [(b, gi) for b in range(NP) for gi in range(NG)]

            EXP_SCALE = float(1.0 / np.sqrt(HD))
            DVE_C0 = float(1024.0 * np.log2(np.e) * EXP_SCALE)
            DVE_C1 = float((15.0 - SCHRAU_C) * 1024.0)

            def emit_scores_exp(b, gi, i):
                # slices 0-1 stage in the ACT ring (stage), slice 2 in its
                # own ring (stgB) consumed by either ACT or DVE - the rings
                # decouple so ACT's slot reuse never waits on the DVE queue.
                grp = groups[gi]
                ns = len(grp)
                na = min(ns, 2)
                qb = q_tiles[b]
                st = stg.tile([128, 2, QB], f32, tag="stage")
                ex = epool.tile([128, GS, QB], f16, tag="ex")
                for slot in range(na):
                    kc, h = grp[slot]
                    mm(st[:, slot, :],
                       kT[h * 64:(h + 1) * 64, kc * 128:(kc + 1) * 128],
                       qb[h * 64:(h + 1) * 64, :],
                       start=True, stop=True)
                stb = None
                if ns == 3:
                    kc, h = grp[2]
                    stb = stg.tile([128, QB], f32, tag="stgB")
                    mm(stb[:],
                       kT[h * 64:(h + 1) * 64, kc * 128:(kc + 1) * 128],
                       qb[h * 64:(h + 1) * 64, :],
                       start=True, stop=True)
                nc.scalar.activation(
                    ex[:, 0:na, :], st[:, 0:na, :], Exp,
                    bias=0.0, scale=EXP_SCALE)
                if ns == 3:
                    # DVE takes the third slice mid-block in steady blocks
                    # (its queue is clumped with normalize work near the
                    # block boundaries); ACT takes it otherwise.
                    off = (i >= NG) and (2 <= gi < NG - 2)
                    if off:
                        nc.vector.tensor_scalar(
                            ex[:, 2, :].bitcast(i16), stb[:],
                            DVE_C0, DVE_C1, MULT, ADD)
                    else:
                        nc.scalar.activation(
                            ex[:, 2, :], stb[:], Exp,
                            bias=0.0, scale=EXP_SCALE)
                return ex

            # normalize block b's ctx accumulators -> cs01 (frees cp ring)
            def emit_normalize(b, ctxp0, ctxp1):
                # h0 normalized in cs01[0:64]; h1 in its own base-0 tile
                # (DVE has no cross-lane path), then DMA'd into cs01[64:].
                cs01 = cpool.tile([128, QB], f16, tag="cs01")
                cs1t = cpool.tile([64, QB], f16, tag="cs1t")
                sums = spool.tile([65, 2 * QB], f16, tag="sums")
                # ACT: denominator rows first (they gate the rb broadcast,
                # whose reciprocal frees the cp ring for block b+1), then
                # the big h0 copy. DVE: h1 copy, then recips BEFORE muls so
                # the cp ring turns over as early as possible.
                nc.scalar.copy(sums[64:65, 0:QB], ctxp0[64:65, :])
                nc.scalar.copy(sums[64:65, QB:2 * QB], ctxp1[64:65, :])
                nc.scalar.copy(cs01[0:64, :], ctxp0[0:64, :])
                nc.vector.tensor_copy(cs1t[:], ctxp1[0:64, :])
                rb0 = cp.tile([64, QB], f32, tag="ctx0")
                mm(rb0[:], ones_t[64:65, :],
                   sums[64:65, 0:QB], start=True, stop=True)
                rb1 = cp.tile([64, QB], f32, tag="ctx1")
                mm(rb1[:], ones_t[64:65, :],
                   sums[64:65, QB:2 * QB], start=True, stop=True)
                rec = spool.tile([64, 2, QB], f32, tag="rec")
                nc.vector.reciprocal_approx_fast(rec[:, 0, :], rb0[:])
                nc.vector.reciprocal_approx_fast(rec[:, 1, :], rb1[:])
                nc.vector.tensor_mul(cs1t[:], cs1t[:], rec[:, 1, :])
                if b < NP - 1:
                    nc.sync.dma_start(cs01[64:128, :], cs1t[:])
                nc.vector.tensor_mul(cs01[0:64, :], cs01[0:64, :],
                                     rec[:, 0, :])
                cs1_box[0] = cs1t
                return cs01

            # one out-proj piece: out[Q*QB + m*128 ... , nh*512 ...]
            TAIL_TAGS = ["stage", "ctx0", "stgB", "ctx1"]
            cs1_box = [None]

            def emit_out_piece(b, cs01, m, nh, pi=0):
                tag = TAIL_TAGS[pi % 4] if b == NP - 1 else "stgB"
                pool = stg if tag in ("stage", "stgB") else cp
                op = pool.tile([128, 512], f32, tag=tag)
                if b == NP - 1:
                    # split per head: no cross-partition cs merge needed
                    mm(op[:], cs01[0:64, m * 128:(m + 1) * 128],
                       wo_t[0:64, nh * 512:(nh + 1) * 512],
                       start=True, stop=False)
                    mm(op[:], cs1_box[0][:, m * 128:(m + 1) * 128],
                       wo2_t[:, nh * 512:(nh + 1) * 512],
                       start=False, stop=True)
                else:
                    mm(op[:], cs01[:, m * 128:(m + 1) * 128],
                       wo_t[:, nh * 512:(nh + 1) * 512], start=True, stop=True)
                ob = opool.tile([128, 512], f16, tag="ob")
                if b == NP - 1 and (m + nh) % 2 == 0:
                    # tail: ACT is done with exp; steal it for half the casts
                    nc.scalar.copy(ob[:], op[:])
                else:
                    nc.vector.tensor_copy(ob[:], op[:])
                nc.sync.dma_start(
                    out_d[b * QB + m * 128:b * QB + (m + 1) * 128,
                          nh * 512:(nh + 1) * 512],
                    ob[:])

            # ---- phase A: block-0 k/q-proj up front; the first two score
            # groups go out before kproj(1) so the exp stream starts ASAP.
            emit_kproj(0, "act")
            emit_qproj(0, "dve")

            # filler schedule: {global ctx iteration: [(when, fn), ...]}
            # when: "pre" runs before the score emission of that iteration
            # (needed for kT producers), "post" runs after it (v tiles etc.)
            fillers = {}

            def add_filler(i, fn, when="pre"):
                fillers.setdefault(i, []).append((when, fn))

            def E(fn, *a, **kw):
                return lambda: fn(*a, **kw)

            # block-0 / block-1 remaining projections. kproj(1) is a pre
            # filler of iteration 0: emitted after the first two score
            # groups (phase A) but before group 2 (which touches kc 4).
            add_filler(0, E(emit_kproj, 1, "act"))
            add_filler(0, E(emit_v, 0, 0), "post")
            add_filler(0, E(emit_v, 0, 1), "post")
            add_filler(1, E(emit_v, 0, 2), "post")
            add_filler(1, E(emit_v, 0, 3), "post")
            add_filler(2, E(emit_qproj, 1, "dve"), "post")
            add_filler(2, E(emit_v, 1, 0), "post")
            add_filler(3, E(emit_v, 1, 1), "post")
            add_filler(3, E(emit_v, 1, 2), "post")
            add_filler(4, E(emit_v, 1, 3), "post")

            # k/v projections of blocks 2..7 during block 0, on their
            # score-frontier deadlines: k-proj(p) must be emitted before the
            # frontier (i + LOOK, group (8p)//3) first touches chunk 4p.
            for p in range(2, NP):
                # scores for group g are emitted at the even iteration
                # i with i+2 <= g <= i+3, so kproj(p) (a pre filler) must
                # land at or before 2*((g-2)//2) for g = (8p)//3.
                base = min(3 * (p - 2) + 4, 2 * (((8 * p) // 3 - 2) // 2))
                add_filler(base, E(emit_kproj, p, "act"))
                for jj in range(4):
                    add_filler(base + 1 + (jj // 2), E(emit_v, p, jj), "post")

            # deferred q-projections: q(p) computed during block p-1; both
            # halves in one iteration (2 stage tiles) to keep ring parity.
            for p in range(2, NP):
                add_filler((p - 1) * NG + 3, E(emit_qproj_h1, p), "post")
                add_filler((p - 1) * NG + 6, E(emit_qproj_h2, p), "post")

            # ---- the flat stream ----
            ex_store = {}
            jbox = [0]

            def emit_scores_upto(lim):
                j = jbox[0]
                while j < len(items) and j <= lim:
                    ex_store[j] = emit_scores_exp(*items[j], j)
                    j += 1
                jbox[0] = j

            emit_scores_upto(LOOK - 1)  # first exps before kproj(1)

            pend_out = []  # deferred out-proj pieces of the previous block
            ctxp0 = ctxp1 = None
            for i, (b, gi) in enumerate(items):
                if gi == 0:
                    ctxp0 = cp.tile([65, QB], f32, tag="ctx0")
                    ctxp1 = cp.tile([65, QB], f32, tag="ctx1")
                pre = [f for w, f in fillers.get(i, ()) if w == "pre"]
                post = [f for w, f in fillers.pop(i, ()) if w == "post"]
                for fn in pre:
                    fn()
                if i % 2 == 0:
                    emit_scores_upto(i + LOOK + 1)
                for fn in post:
                    fn()
                # deferred epilogue pieces of the previous q-block,
                # drained in PAIRS so the stage-ring parity of the score
                # stream is preserved.
                if pend_out and gi >= 2 and gi % 2 == 0:
                    pb, pcs, pm, pnh = pend_out.pop(0)
                    emit_out_piece(pb, pcs, pm, pnh)
                # ctx accumulation for group gi
                ex = ex_store.pop(i)
                for slot, (kc, h) in enumerate(groups[gi]):
                    ctxp = ctxp0 if h == 0 else ctxp1
                    mm(ctxp[:], v_tiles[kc][:, h, :], ex[:, slot, :],
                       start=(kc == 0), stop=(kc == KC - 1))
                if gi == NG - 1:
                    # normalize now (frees ctx ring for b+1); out-proj
                    # pieces trail into the next block's groups.
                    cs01 = emit_normalize(b, ctxp0, ctxp1)
                    pieces = [(b, cs01, m, nh)
                              for m in range(QB // 128)
                              for nh in range(D // 512)]
                    if b + 1 < NP:
                        pend_out.extend(pieces)
                    else:
                        for pi, (pb, pcs, pm, pnh) in enumerate(pieces):
                            emit_out_piece(pb, pcs, pm, pnh, pi)
            # flush any stragglers
            for pb, pcs, pm, pnh in pend_out:
                emit_out_piece(pb, pcs, pm, pnh)

    nc.compile()
    return nc


def _shard_inputs(x, wq, bq, wk, bk, wv, bv, wo, bo, s):
    npdt16 = np.float16
    # [D, s] -> contiguous per-block layout [s//512, 128, D//128, 512]
    xT2 = np.asarray(x, np.float32).reshape(s, D).T
    xT = np.ascontiguousarray(
        xT2.reshape(D // 128, 128, s // 512, 512).transpose(2, 1, 0, 3)
    ).astype(npdt16)

    def lhsT_layout(w, c):
        blk = np.asarray(w, np.float32)[:, c * 128:(c + 1) * 128]
        return np.ascontiguousarray(
            blk.reshape(DC, 128, 128).transpose(1, 0, 2).reshape(128, D)
        ).astype(npdt16)

    def wv_aug_layout(w, c):
        # [128, DC, 130]: per d-chunk, [h0 cols | 0 | h1 cols | 0]
        blk = np.asarray(w, np.float32)[:, c * 128:(c + 1) * 128]  # [D, 128]
        aug = np.zeros((DC, 128, 130), np.float32)
        aug[:, :, 0:64] = blk[:, 0:64].reshape(DC, 128, 64)
        aug[:, :, 65:129] = blk[:, 64:128].reshape(DC, 128, 64)
        return np.ascontiguousarray(aug.transpose(1, 0, 2)).astype(npdt16)

    def bvb_layout(bv, c):
        # [128, 2, 65]: v bias broadcast over k-rows + ones column
        bvc = np.asarray(bv, np.float32)[c * 128:(c + 1) * 128]
        t = np.empty((2, 65), np.float32)
        t[0, 0:64] = bvc[0:64]
        t[1, 0:64] = bvc[64:128]
        t[:, 64] = 1.0
        return np.ascontiguousarray(
            np.broadcast_to(t, (128, 2, 65))).astype(npdt16)

    in_maps = []
    for c in range(N_CORES):
        in_maps.append({
            "xT": xT,
            "wq": lhsT_layout(wq, c),
            "wk": lhsT_layout(wk, c),
            "wv": wv_aug_layout(wv, c),
            "bq": np.ascontiguousarray(
                np.asarray(bq, np.float32)[c * 128:(c + 1) * 128, None]),
            "bk": np.ascontiguousarray(
                np.asarray(bk, np.float32)[c * 128:(c + 1) * 128, None]),
            "bvb": bvb_layout(bv, c),
            "wo": np.ascontiguousarray(
                np.asarray(wo, np.float32)[c * 128:(c + 1) * 128, :]
            ).astype(npdt16),
            "wo2": np.ascontiguousarray(
                np.asarray(wo, np.float32)[c * 128 + 64:(c + 1) * 128, :]
            ).astype(npdt16),
        })
    return in_maps


def run(x, wq, bq, wk, bk, wv, bv, wo, bo, trace=False, s=S):
    global _LAST_EXEC_NS
    from concourse.bass_utils import run_bass_kernel_spmd

    if trace:
        _install_ntff_hook_shim()
    nc = _build(s)
    in_maps = _shard_inputs(x, wq, bq, wk, bk, wv, bv, wo, bo, s)
    res = run_bass_kernel_spmd(nc, in_maps, core_ids=list(range(N_CORES)),
                               trace=trace)
    _LAST_EXEC_NS = res.exec_time_ns
    out = res.results[0]["out"].astype(np.float64)
    for c in range(1, N_CORES):
        out += res.results[c]["out"]
    out += np.asarray(bo, np.float64)
    return out.astype(np.float32).reshape(1, s, D)


def kernel(x, wq, bq, wk, bk, wv, bv, wo, bo):
    trace = bool(os.environ.get("BASS_MHA_TRACE"))
    return run(x, wq, bq, wk, bk, wv, bv, wo, bo, trace=trace)
